# revision 1
# baseline (speedup 1.0000x reference)
"""NEAT layer kernel for Trainium2 (8 NeuronCores, pure data parallel).

Math (per reference): vals starts as x [B,64]; for each layer li with
(src, w, b): z = sum_k vals[:, src[n,k]] * w[n,k] + b[n]; out = sigmoid(5*z);
vals = concat(vals, out). Output = layer-3 out [B,10].

Strategy: the sparse gather+einsum is a sparse matmul over the accumulated
node values. Host-side we scatter the tiny (n,16) weights into dense
[src_block, dest_nodes] matrices, then on-device it is a chain of dense
matmuls (nodes on partitions, batch on the free dim) + fused
sigmoid(5z+5b) activations. z2 (32 nodes, PSUM rows 0..31) and the z3
partial (10 nodes, rows 32..41) are packed into one PSUM tile [64, N]
so their shared source blocks (x, out0, out1) need a single matmul
each: 8 matmuls per 512-sample chunk.

All matmul operands use float32r (fp32 rounded to 11 mantissa bits;
1 cyc/row on the PE vs 4 for fp32). The walrus birverifier requires
every producer of an fp32r matmul operand to write with fp32r dtype and
every fp32r matmul dst to start at partition 0 — hence the host-side
pre-rounding of x/weights DMA'd in as fp32r, fp32r ACT outputs, and the
z3-at-row-0 packing with O2/Wh23 at base partition 32.

All inputs (weights, biases, x) travel in ONE [128, 4622] fp32r blob
moved by a single DMA: the HW codegen supports only one sync-wait slot
per fused-fp32r Matmult and per Activation, so every instruction must
depend on at most one semaphore lane (same-engine Act->Act deps also
cost a wait slot because the Act pipeline overlaps instructions).
x [64, 8192] is folded to [128, 4096] (second half of the batch on
partitions 64..127, with the x-weight blocks duplicated there); biases
ride in 4 fp32 columns read directly as ACT bias APs via bitcast, with
one warmup Act copy so the blob-DMA wait enters the Act engine clock.
z3 gets its own PSUM bank: its x/out0/out1 partial (rows 0..9 of the
packed p23 tile) is copied next to out2 in SBUF and folded into the
Wh23 matmul through 10 identity rows, so the final o3 sigmoid reads a
bank written only by the PE (single PE wait, no Act->Act dep).

Batch 65536 is split 8192 per core; each core runs 16 chunks of 512.
"""

import sys

sys.path.insert(0, "/opt/trn_rl_repo")

import numpy as np

import concourse.bass as bass
import concourse.mybir as mybir
from concourse.tile import TileContext

BATCH = 65536
IN_DIM = 64
FAN_IN = 16
GAIN = 5.0
N_CORES = 8
BC = BATCH // N_CORES          # 8192 samples per core
CHUNK = 512
N_CHUNKS = BC // CHUNK         # 16

# Node index blocks in the accumulated `vals` array.
X_LO, X_HI = 0, 64             # x block
H0_LO, H0_HI = 64, 192         # out0 block
H1_LO, H1_HI = 192, 288        # out1 block
H2_LO, H2_HI = 288, 320        # out2 block

F32 = mybir.dt.float32
F32R = mybir.dt.float32r

# Blob column layout (128 partitions x NB fp32r words).
OFF_WX0 = 0        # [64,128] x->l0, duplicated on partitions 64..127
OFF_WX1 = 128      # [64,96]  x->l1, duplicated
OFF_WX23 = 224     # [64,64]  x->l3(cols 0..9)+l2(cols 32..63), duplicated
OFF_WH01 = 288     # [128,96] out0->l1
OFF_WH023 = 384    # [128,64] out0->l3/l2 packed
OFF_WH123 = 448    # [96,64]  out1->l3/l2 packed
OFF_WH23 = 512     # [32,10] at partitions 32..63: out2->l3
OFF_B = 522        # 4 fp32 cols: 5*b0(r0..127), 5*b1(r0..95), 5*b2(r32..63), 5*b3(r0..9)
OFF_X = 526        # [128, 4096]: xT cols 0..4095 on r0..63, cols 4096..8191 on r64..127
NB = OFF_X + BC // 2           # 4622


def _round_fp32r(a: np.ndarray) -> np.ndarray:
    """Round fp32 to fp32r (RNE to 11 explicit mantissa bits, low 12 bits 0)."""
    u = np.ascontiguousarray(a, dtype=np.float32).view(np.uint32)
    lsb = (u >> 12) & 1
    r = (u + 0x7FF + lsb) & 0xFFFFF000
    return r.view(np.float32)


def _scatter(dst: np.ndarray, src: np.ndarray, w: np.ndarray, lo: int, hi: int,
             col_off: int) -> None:
    """dst[src[n,k]-lo, n+col_off] += w[n,k] for src entries in [lo,hi)."""
    n, k = src.shape
    cols = np.repeat(np.arange(n, dtype=np.int64), k) + col_off
    s = src.ravel().astype(np.int64)
    v = w.ravel().astype(np.float64)
    m = (s >= lo) & (s < hi)
    np.add.at(dst, (s[m] - lo, cols[m]), v[m])


def _build_blob_base(inputs: dict) -> np.ndarray:
    """Weights+biases portion of the blob (x region left zero)."""
    Wx0 = np.zeros([64, 128], np.float64)
    Wx1 = np.zeros([64, 96], np.float64)
    Wx23 = np.zeros([64, 64], np.float64)
    Wh01 = np.zeros([128, 96], np.float64)
    Wh023 = np.zeros([128, 64], np.float64)
    Wh123 = np.zeros([96, 64], np.float64)
    Wh23 = np.zeros([32, 10], np.float64)

    # z23 packing: z2 dest nodes at cols 0..31, z3 partial at cols 32..41.
    _scatter(Wx0, inputs["src0"], inputs["w0"], X_LO, X_HI, 0)

    _scatter(Wx1, inputs["src1"], inputs["w1"], X_LO, X_HI, 0)
    _scatter(Wh01, inputs["src1"], inputs["w1"], H0_LO, H0_HI, 0)

    _scatter(Wx23, inputs["src2"], inputs["w2"], X_LO, X_HI, 0)
    _scatter(Wh023, inputs["src2"], inputs["w2"], H0_LO, H0_HI, 0)
    _scatter(Wh123, inputs["src2"], inputs["w2"], H1_LO, H1_HI, 0)

    _scatter(Wx23, inputs["src3"], inputs["w3"], X_LO, X_HI, 32)
    _scatter(Wh023, inputs["src3"], inputs["w3"], H0_LO, H0_HI, 32)
    _scatter(Wh123, inputs["src3"], inputs["w3"], H1_LO, H1_HI, 32)
    _scatter(Wh23, inputs["src3"], inputs["w3"], H2_LO, H2_HI, 0)

    blob = np.zeros([128, NB], np.float32)
    for half in (slice(0, 64), slice(64, 128)):
        blob[half, OFF_WX0:OFF_WX0 + 128] = _round_fp32r(Wx0)
        blob[half, OFF_WX1:OFF_WX1 + 96] = _round_fp32r(Wx1)
        blob[half, OFF_WX23:OFF_WX23 + 64] = _round_fp32r(Wx23)
    blob[0:128, OFF_WH01:OFF_WH01 + 96] = _round_fp32r(Wh01)
    blob[0:128, OFF_WH023:OFF_WH023 + 64] = _round_fp32r(Wh023)
    blob[0:96, OFF_WH123:OFF_WH123 + 64] = _round_fp32r(Wh123)
    blob[0:32, OFF_WH23:OFF_WH23 + 10] = _round_fp32r(Wh23)
    # Identity rows: fold the z3 partial (copied to partitions 32..41 of
    # the out2 tile) into the Wh23 matmul.
    blob[32:42, OFF_WH23:OFF_WH23 + 10] = np.eye(10, dtype=np.float32)

    blob[0:128, OFF_B + 0] = GAIN * np.asarray(inputs["b0"], np.float32)
    blob[0:96, OFF_B + 1] = GAIN * np.asarray(inputs["b1"], np.float32)
    blob[0:32, OFF_B + 2] = GAIN * np.asarray(inputs["b2"], np.float32)
    blob[0:10, OFF_B + 3] = GAIN * np.asarray(inputs["b3"], np.float32)
    return blob


def build_nc() -> bass.Bass:
    nc = bass.Bass()
    blob_d = nc.declare_dram_parameter("blob", [128, NB], F32R, isOutput=False)
    # Four output tensors so partial results stream out overlapped with
    # compute: separate DRAM tensors avoid the WAW-serialization waits a
    # shared output tensor would add to each DMA (one sync wait allowed).
    yTs = [nc.declare_dram_parameter(f"yT{k}", [10, BC // 4], F32,
                                     isOutput=True) for k in range(4)]

    SIG = mybir.ActivationFunctionType.Sigmoid

    with TileContext(nc) as tc:
        with (
            tc.tile_pool(name="persist", bufs=1) as pp,
            tc.tile_pool(name="ps0", bufs=2, space="PSUM") as pz0,
            tc.tile_pool(name="ps1", bufs=2, space="PSUM") as pz1,
            tc.tile_pool(name="ps23", bufs=2, space="PSUM") as pz23,
            tc.tile_pool(name="psq", bufs=2, space="PSUM") as pzq,
        ):
            blob_sb = pp.tile([128, NB], F32R)
            warm_sb = pp.tile([128, 1], F32)
            o0_sb = pp.tile([128, BC], F32R)
            o1_sb = pp.tile([96, BC], F32R)
            # out2 on partitions 0..31, z3 partial copy on 32..41.
            o2_sb = pp.tile([42, BC], F32R)
            o3_sb = pp.tile([10, BC], F32)

            def bias(lo, hi, j):
                return blob_sb[lo:hi, OFF_B + j:OFF_B + j + 1].bitcast(F32)

            nc.sync.dma_start(out=blob_sb[:], in_=blob_d[:])
            # Warmup: puts the blob-DMA wait into the Act engine clock so
            # later ACTs' blob deps are elided (1 wait slot each).
            nc.scalar.copy(warm_sb[:], blob_sb[:, OFF_B:OFF_B + 1].bitcast(F32))

            # Software pipeline, layer-major: at step t chunk t runs layer 0
            # while chunk t-1 runs layer 1, t-2 layer 2, t-3 layer 3. The
            # per-chunk serial chain mm->act->mm->act no longer head-of-line
            # blocks the PE: each step's independent matmuls of younger
            # chunks are emitted before the older chunks' dependent ones,
            # so PE stalls only if the Act engine falls a full step behind.
            # Per-layer PSUM pools (1 bank x 2 bufs each = 8 banks) keep
            # 3 chunks in flight. Wait-slot audit (1 sync wait max per
            # instruction): in steady state only M3 (p1+=Wh01@O0, waits
            # A1(c1)) takes a new Act wait on the PE; every other matmul's
            # dep value is below the PE engine clock. ACTs wait their
            # producing matmul only (bias deps elided via the warmup copy).
            def xslc(c):
                rb = 0 if c < 8 else 64
                xc = OFF_X + (c % 8) * CHUNK
                return rb, blob_sb[rb:rb + 64, xc:xc + CHUNK]

            p0s, p1s, p23s, q3s = {}, {}, {}, {}
            for t in range(N_CHUNKS + 3):
                c0, c1, c2, c3 = t, t - 1, t - 2, t - 3
                if c0 < N_CHUNKS:
                    cs = slice(c0 * CHUNK, (c0 + 1) * CHUNK)
                    rb, X = xslc(c0)
                    p0 = p0s[c0] = pz0.tile([128, CHUNK], F32, name="p0")
                    nc.tensor.matmul(p0[:], blob_sb[rb:rb + 64, 0:128], X,
                                     start=True, stop=True)
                    nc.scalar.activation(o0_sb[:, cs], p0[:], SIG,
                                         bias=bias(0, 128, 0), scale=GAIN)
                if 0 <= c1 < N_CHUNKS:
                    cs = slice(c1 * CHUNK, (c1 + 1) * CHUNK)
                    rb, X = xslc(c1)
                    p1 = p1s[c1] = pz1.tile([96, CHUNK], F32, name="p1")
                    nc.tensor.matmul(p1[:], blob_sb[rb:rb + 64, 128:224], X,
                                     start=True, stop=False)
                    nc.tensor.matmul(p1[:], blob_sb[0:128, 288:384],
                                     o0_sb[:, cs], start=False, stop=True)
                    nc.scalar.activation(o1_sb[:, cs], p1[:], SIG,
                                         bias=bias(0, 96, 1), scale=GAIN)
                if 0 <= c2 < N_CHUNKS:
                    cs = slice(c2 * CHUNK, (c2 + 1) * CHUNK)
                    rb, X = xslc(c2)
                    p23 = p23s[c2] = pz23.tile([64, CHUNK], F32, name="p23")
                    nc.tensor.matmul(p23[:], blob_sb[rb:rb + 64, 224:288], X,
                                     start=True, stop=False)
                    nc.tensor.matmul(p23[:], blob_sb[0:128, 384:448],
                                     o0_sb[:, cs], start=False, stop=False)
                    nc.tensor.matmul(p23[:], blob_sb[0:96, 448:512],
                                     o1_sb[:, cs], start=False, stop=True)
                    nc.scalar.activation(o2_sb[0:32, cs], p23[0:32, :], SIG,
                                         bias=bias(0, 32, 2), scale=GAIN)
                    nc.scalar.copy(o2_sb[32:42, cs], p23[32:42, :])
                if 0 <= c3 < N_CHUNKS:
                    cs = slice(c3 * CHUNK, (c3 + 1) * CHUNK)
                    q3 = q3s[c3] = pzq.tile([10, CHUNK], F32, name="q3")
                    nc.tensor.matmul(q3[:], blob_sb[0:42, 512:522],
                                     o2_sb[0:42, cs], start=True, stop=True)
                    nc.scalar.activation(o3_sb[:, cs], q3[:], SIG,
                                         bias=bias(0, 10, 3), scale=GAIN)
                    if c3 % 4 == 3:
                        k = c3 // 4
                        g = slice(k * (BC // 4), (k + 1) * (BC // 4))
                        nc.sync.dma_start(out=yTs[k][:], in_=o3_sb[:, g])

    # The teardown Drain waits on every engine's final sem value, but HW
    # allows one sync wait per instruction. The LAST out DMA's completion
    # transitively dominates them all: it starts after the final ACT
    # (which waited the final matmul, which waited the blob DMA), and all
    # DMAs share one FIFO queue so the earlier out DMAs finish before it.
    # Prune the drain to that lane alone.
    for i in nc.all_instructions():
        if type(i).__name__ == "InstDrain" and i.sync_info and \
                len(i.sync_info.on_wait) > 1:
            dma_lane = None
            for j in nc.all_instructions():
                if type(j).__name__ == "InstDMACopy" and j.sync_info:
                    for u in j.sync_info.on_update:
                        if j.sync_info.on_wait:
                            dma_lane = u.ant_name
            si = i.sync_info
            si.on_wait = [w for w in si.on_wait if w.ant_name == dma_lane]
            i.sync_info = si
    return nc


def make_in_maps(inputs: dict) -> list[dict]:
    base = _build_blob_base(inputs)
    x = _round_fp32r(np.asarray(inputs["x"], dtype=np.float32))
    in_maps = []
    for i in range(N_CORES):
        b = base.copy()
        xT = np.ascontiguousarray(x[i * BC:(i + 1) * BC, :].T)  # [64, 8192]
        b[0:64, OFF_X:OFF_X + BC // 2] = xT[:, 0:BC // 2]
        b[64:128, OFF_X:OFF_X + BC // 2] = xT[:, BC // 2:]
        in_maps.append({"blob": b})
    return in_maps


def assemble_output(results: list[dict]) -> np.ndarray:
    y = np.empty((BATCH, 10), np.float32)
    for i in range(N_CORES):
        yT = np.concatenate([results[i][f"yT{k}"] for k in range(4)], axis=1)
        y[i * BC:(i + 1) * BC, :] = yT.T
    return y


def kernel(**inputs: np.ndarray) -> np.ndarray:
    from concourse.bass_utils import run_bass_kernel_spmd

    nc = build_nc()
    in_maps = make_in_maps(inputs)
    res = run_bass_kernel_spmd(nc, in_maps, list(range(N_CORES)))
    return assemble_output(res.results)

